# revision 27
# baseline (speedup 1.0000x reference)
"""Trainium2 Bass kernel for nn_Attention_4_lora (B=8, T=1024, C=1024, R=64).

Strategy: data-parallel over the batch dim (1 batch per NeuronCore, 8 cores).
All activations live in transposed [channel, token] layout so that every
matmul contraction runs over the SBUF partition axis. BatchNorm statistics
are reduced across cores with two small AllReduces. All heavy matmuls
run in float32r (TF32-like, full PE throughput at free-dim >= 256).

Weight merging (vs the previous revision): Wm^T = W^T + delta^T with
delta^T[c, 3e+f] = sum_r lbB[r, 1024f+c] * laT[r, e]. Weights are merged in
768-wide d-groups (768 = 3*256), so group g consumes exactly the contiguous
e-range [256g, 256g+256): the delta matmuls run with a 256-wide free dim
(full PE rate; the old 171-wide quarters paid the 4x f32r penalty) and land
in PSUM, then a single stride-3 DVE add folds each f-phase into the slab.

Per-core pipeline:
  P1  per d-group: slab DMA + 24 delta matmuls + 24 strided adds
  P2  xa^T[d, t] = Wm^T-slab.T @ x^T for q,k channels + bn_stats per tile
      (d-groups 0..1 and the k-tail of group 2)
  P3  v[t, c] natural layout (groups 2..3) + ones-matmul stats
  P4  AllReduce #1 (q,k stats) issued after the k-tail, hidden behind the
      v pass; AllReduce #2 (v stats) hidden behind P5
  P5  scores^T[s, t] = k^T-slab.T @ q^T, exp((q.k)/32) on ScalarE,
      causal mask via affine_select, row sums via ones-matmul; 1/r is
      computed and broadcast per 512-token chunk so the P6 drain never
      waits on the full row
  P6  y^T[c, t] = v-slab.T @ att_exp^T, fused 1/r + BN affine on drain
  P7  y1^T = Wp^T-slab.T @ y^T ; y2^T = Wmp^T-slab.T @ y1^T -> out [C, T]
      (final drain DMAs straight from PSUM to DRAM)

kernel() takes the full unsharded inputs, shards/uploads, runs SPMD on
cores 0-7, gathers, and transposes back to [B, T, C].
"""

import numpy as np

import concourse.bass as bass
import concourse.mybir as mybir
import concourse.tile as tile
from concourse import bacc
from concourse.bass_utils import run_bass_kernel_spmd

NCORES = 8
C = 1024
R = 64
D3 = 3 * C
EPS = 1e-5
F32 = mybir.dt.float32
F32R = mybir.dt.float32r
BF16 = mybir.dt.bfloat16
AX = mybir.AxisListType
OP = mybir.AluOpType
ACTF = mybir.ActivationFunctionType

NG = 4          # weight d-groups
GW = 768        # d-width per group (= 3 * EW)
EW = 256        # e-width per group


def build(T=1024, single_core=False, no_collective=False, reps=1):
    assert T % 512 == 0

    nc = bacc.Bacc(None, target_bir_lowering=False,
                   num_devices=(1 if single_core else NCORES))

    prm = {}
    prm["xT"] = nc.declare_dram_parameter("xT", [C, T], F32R, isOutput=False)
    prm["wT"] = nc.declare_dram_parameter("wT", [C, D3], F32R, isOutput=False)
    prm["wpT"] = nc.declare_dram_parameter("wpT", [C, C], F32R, isOutput=False)
    prm["laT"] = nc.declare_dram_parameter("laT", [R, C], F32R, isOutput=False)
    prm["lbB"] = nc.declare_dram_parameter("lbB", [R, D3], F32R, isOutput=False)
    prm["lpaT"] = nc.declare_dram_parameter("lpaT", [R, C], F32R, isOutput=False)
    prm["lpbB"] = nc.declare_dram_parameter("lpbB", [R, C], F32R, isOutput=False)
    prm["gam"] = nc.declare_dram_parameter("gam", [D3], F32, isOutput=False)
    prm["bet"] = nc.declare_dram_parameter("bet", [D3], F32, isOutput=False)
    prm["out"] = nc.declare_dram_parameter("out", [C, T], F32, isOutput=True)

    with tile.TileContext(nc) as tc:
        for rep in range(reps):
            _emit(nc, tc, prm, T, rep, single_core, no_collective)

    nc.compile()
    return nc


def _emit(nc, tc, prm, T, rep, single_core, no_collective):
    NT = T // 128
    TQ = T // 512
    xT, wT, wpT, laT, lbB = prm["xT"], prm["wT"], prm["wpT"], prm["laT"], prm["lbB"]
    lpaT, lpbB, gam, bet, out = prm["lpaT"], prm["lpbB"], prm["gam"], prm["bet"], prm["out"]

    stats_in = nc.dram_tensor(f"stats_in_{rep}", [4096], F32)
    stats_out = nc.dram_tensor(f"stats_out_{rep}", [4096], F32)
    vstats_in = nc.dram_tensor(f"vstats_in_{rep}", [2 * C], F32)
    vstats_out = nc.dram_tensor(f"vstats_out_{rep}", [2 * C], F32)
    rb_dram = nc.dram_tensor(f"rb_{rep}", [T], F32)

    def bcast_dram(param, offset, n):
        return bass.AP(tensor=param[:].tensor, offset=offset, ap=[[0, 128], [1, n]])

    def emit_rsqrt(dst, var_t, u_t, r_t, iters=14):
        """dst = 1/sqrt(var + EPS), DVE only. u = 1/(var+eps) via the
        blessed vector reciprocal, then Babylonian iteration for sqrt(u):
        y' = 0.5*(y + u/y), seeded with 0.5*(1+u)."""
        nc.vector.tensor_scalar(out=u_t, in0=var_t, scalar1=EPS, scalar2=None,
                                op0=OP.add)
        nc.vector.reciprocal(out=u_t, in_=u_t)
        nc.vector.tensor_scalar(out=dst, in0=u_t, scalar1=0.5, scalar2=0.5,
                                op0=OP.mult, op1=OP.add)
        for _ in range(iters):
            nc.vector.reciprocal(out=r_t, in_=dst)
            nc.vector.tensor_tensor(out=r_t, in0=r_t, in1=u_t, op=OP.mult)
            nc.vector.tensor_tensor(out=dst, in0=dst, in1=r_t, op=OP.add)
            nc.vector.tensor_scalar(out=dst, in0=dst, scalar1=0.5, scalar2=None,
                                    op0=OP.mult)

    def allreduce(ins, outs):
        if single_core or no_collective:
            nc.sync.dma_start(out=outs, in_=ins)
        else:
            nc.gpsimd.collective_compute(
                "AllReduce", OP.add,
                replica_groups=[list(range(NCORES))],
                ins=[ins], outs=[outs])

    with (
        tc.tile_pool(name=f"misc{rep}", bufs=1) as misc,
        tc.tile_pool(name=f"outst{rep}", bufs=3) as outst,
        tc.tile_pool(name=f"vpool{rep}", bufs=1) as vpool,
        tc.tile_pool(name=f"attp{rep}", bufs=1) as attp,
        tc.tile_pool(name=f"psA{rep}", bufs=5, space="PSUM") as psA,
    ):
        # ---------------- constants / act-table warmup ----------------
        ones_f = misc.tile([128, 1], F32)
        nc.vector.memset(ones_f[:, :], 1.0)
        ones_r = misc.tile([128, 1], F32R)
        nc.vector.tensor_copy(out=ones_r[:, :], in_=ones_f[:, :])
        ones_b = misc.tile([128, 1], BF16)
        nc.vector.tensor_copy(out=ones_b[:, :], in_=ones_f[:, :])

        # Every activation below uses a func whose canonical table set is
        # exp_and_others (Exp/Copy/Square), so the act table is loaded
        # exactly once; warm it here, off the critical path. rsqrt for the
        # BN stats is computed on the DVE (Babylonian sqrt of 1/x) so the
        # Sqrt/Ln table sets are never touched.
        warm = misc.tile([1, 2], F32)
        nc.vector.memset(warm[0:1, :], 1.0)
        nc.scalar.activation(out=warm[0:1, :], in_=warm[0:1, :], func=ACTF.Exp)

        qk_mv = misc.tile([128, 16, 2], F32)
        m16 = misc.tile([128, 16], F32)
        qa = misc.tile([128, 16], F32)
        qb = misc.tile([128, 16], F32)
        r_bc = misc.tile([128, T], F32)

        xa = [None] * 16
        vnat = [None] * NT

        with tc.tile_pool(name=f"xapool{rep}", bufs=1) as xapool:
            with tc.tile_pool(name=f"lorap{rep}", bufs=1) as lorap:
                la_sb = lorap.tile([R, C], F32R)
                nc.sync.dma_start(out=la_sb[:, :], in_=laT[:, :])
                # f=0 third first so the first delta matmuls start ~2.5us in
                lb_sb = lorap.tile([R, D3], F32R)
                for _c in range(3):
                    nc.sync.dma_start(out=lb_sb[:, 1024 * _c:1024 * (_c + 1)],
                                      in_=lbB[:, 1024 * _c:1024 * (_c + 1)])

                with tc.tile_pool(name=f"xtpool{rep}", bufs=1) as xtpool:
                    with tc.tile_pool(name=f"wb{rep}", bufs=1) as wbp:

                        def merge_group(g):
                            """Merged Wm^T[:, 768g:768(g+1)] as 8 c-tiles
                            [128, 768]; delta matmuls straight from PSUM."""
                            d0 = GW * g
                            wq = []
                            for ct in range(8):
                                w_t = wbp.tile([128, GW], F32R, tag=f"wb{ct}",
                                               bufs=2, name=f"wq{g}_{ct}")
                                nc.sync.dma_start(
                                    out=w_t[:, :],
                                    in_=wT[128 * ct:128 * (ct + 1), d0:d0 + GW])
                                wq.append(w_t)
                            for f in range(3):
                                for ct in range(8):
                                    view3 = wq[ct][:, :].rearrange(
                                        "p (u three) -> p u three", three=3)
                                    ps = psA.tile([128, 512], F32, tag="mm",
                                                  name=f"dps{g}_{ct}_{f}")
                                    nc.tensor.matmul(
                                        ps[:, 0:EW],
                                        lb_sb[:, 1024 * f + 128 * ct:
                                              1024 * f + 128 * (ct + 1)],
                                        la_sb[:, EW * g:EW * (g + 1)],
                                        start=True, stop=True)
                                    nc.vector.tensor_tensor(
                                        out=view3[:, :, f],
                                        in0=view3[:, :, f],
                                        in1=ps[:, 0:EW], op=OP.add)
                            return wq

                        wq = merge_group(0)
                        xt = []
                        for k in range(8):
                            x_t = xtpool.tile([128, T], F32R, tag=f"xt{k}",
                                              name=f"xt{k}")
                            nc.sync.dma_start(out=x_t[:, :],
                                              in_=xT[128 * k:128 * (k + 1), :])
                            xt.append(x_t)

                        gqk = misc.tile([128, 16], F32)
                        nc.sync.dma_start(
                            out=gqk[:, :],
                            in_=gam[0:2048].rearrange("(i p) -> p i", p=128))
                        bqk = misc.tile([128, 16], F32)
                        nc.sync.dma_start(
                            out=bqk[:, :],
                            in_=bet[0:2048].rearrange("(i p) -> p i", p=128))

                        with tc.tile_pool(name=f"psV{rep}", bufs=1,
                                          space="PSUM") as psV:
                            for g in range(NG):
                                if g > 0:
                                    wq = merge_group(g)
                                # ---- q,k channels of this group ----
                                for il_local in range(6):
                                    il = 6 * g + il_local
                                    if il >= 16:
                                        break
                                    xa_g = xapool.tile([128, T], F32R,
                                                       tag=f"xa{il}",
                                                       name=f"xa{il}")
                                    for tch in range(TQ):
                                        ps = psA.tile([128, 512], F32, tag="mm",
                                                      name=f"xaps{il}_{tch}")
                                        for k in range(8):
                                            nc.tensor.matmul(
                                                ps[:, :],
                                                wq[k][:, 128 * il_local:
                                                      128 * (il_local + 1)],
                                                xt[k][:, 512 * tch:512 * (tch + 1)],
                                                start=(k == 0), stop=(k == 7))
                                        nc.scalar.copy(
                                            out=xa_g[:, 512 * tch:512 * (tch + 1)],
                                            in_=ps[:, :])
                                    bnstat = misc.tile([128, TQ, 6], F32,
                                                       tag="bnstat", bufs=2,
                                                       name=f"bnstat{il}")
                                    for j in range(TQ):
                                        nc.vector.bn_stats(
                                            out=bnstat[:, j, :],
                                            in_=xa_g[:, 512 * j:512 * (j + 1)])
                                    nc.vector.bn_aggr(out=qk_mv[:, il, :],
                                                      in_=bnstat[:, :, :])
                                    xa[il] = xa_g

                                if g == 2:
                                    # qk stats -> (mean, E[x^2]) packed -> AR1
                                    nc.vector.tensor_tensor(
                                        out=m16[:, :], in0=qk_mv[:, :, 0],
                                        in1=qk_mv[:, :, 0], op=OP.mult)
                                    nc.vector.tensor_tensor(
                                        out=qk_mv[:, :, 1], in0=qk_mv[:, :, 1],
                                        in1=m16[:, :], op=OP.add)
                                    nc.sync.dma_start(
                                        out=stats_in[0:4096].rearrange(
                                            "(p i s) -> p i s", p=128, s=2),
                                        in_=qk_mv[:, :, :])
                                    allreduce(stats_in[:], stats_out[:])

                                # ---- v channels of this group ----
                                # group g covers d in [768g, 768g+768); v is
                                # d in [2048, 3072) -> slab cols [vc0, 768)
                                d0 = GW * g
                                vc0 = max(0, 2048 - d0)
                                if vc0 >= GW:
                                    continue
                                chunks = []
                                c0 = vc0
                                while c0 < GW:
                                    w = min(512, GW - c0)
                                    chunks.append((c0, w))
                                    c0 += w
                                for (c0, w) in chunks:
                                    vbase = d0 + c0 - 2048
                                    ps_vs = psV.tile([1, 512], F32, tag="vs",
                                                     bufs=1, name=f"psvs{g}_{c0}")
                                    ps_vq = psV.tile([1, 512], F32, tag="vq",
                                                     bufs=1, name=f"psvq{g}_{c0}")
                                    for tt in range(NT):
                                        if vnat[tt] is None:
                                            vnat[tt] = vpool.tile(
                                                [128, C], BF16,
                                                tag=f"v{tt}", name=f"v{tt}")
                                        ps = psA.tile([128, 512], F32, tag="mm",
                                                      name=f"vps{g}_{c0}_{tt}")
                                        for k in range(8):
                                            nc.tensor.matmul(
                                                ps[:, 0:w],
                                                xt[k][:, 128 * tt:128 * (tt + 1)],
                                                wq[k][:, c0:c0 + w],
                                                start=(k == 0), stop=(k == 7))
                                        vsl = vnat[tt][:, vbase:vbase + w]
                                        nc.scalar.copy(out=vsl, in_=ps[:, 0:w])
                                        sq = misc.tile([128, 512], F32R, tag="sq",
                                                       bufs=1,
                                                       name=f"sq{g}_{c0}_{tt}")
                                        nc.scalar.activation(
                                            out=sq[:, 0:w], in_=vsl,
                                            func=ACTF.Square)
                                        nc.tensor.matmul(
                                            ps_vs[0:1, 0:w], ones_b[:, :], vsl,
                                            start=(tt == 0), stop=(tt == NT - 1))
                                        nc.tensor.matmul(
                                            ps_vq[0:1, 0:w], ones_r[:, :],
                                            sq[:, 0:w],
                                            start=(tt == 0), stop=(tt == NT - 1))
                                    vst1 = misc.tile([1, 512], F32, tag="vst",
                                                     bufs=1, name=f"vst1_{g}_{c0}")
                                    nc.vector.tensor_copy(out=vst1[0:1, 0:w],
                                                          in_=ps_vs[0:1, 0:w])
                                    nc.sync.dma_start(
                                        out=vstats_in[vbase:vbase + w],
                                        in_=vst1[0:1, 0:w])
                                    vst2 = misc.tile([1, 512], F32, tag="vst",
                                                     bufs=1, name=f"vst2_{g}_{c0}")
                                    nc.vector.tensor_copy(out=vst2[0:1, 0:w],
                                                          in_=ps_vq[0:1, 0:w])
                                    nc.sync.dma_start(
                                        out=vstats_in[C + vbase:C + vbase + w],
                                        in_=vst2[0:1, 0:w])
                            allreduce(vstats_in[:], vstats_out[:])

                # ---- AR1 readback: q,k normalization (emitted after all
                # group-loop DVE work so the wait doesn't stall the queue) ----
                ar_qk = misc.tile([128, 16, 2], F32)
                nc.sync.dma_start(
                    out=ar_qk[:, :, :],
                    in_=stats_out[0:4096].rearrange("(p i s) -> p i s",
                                                    p=128, s=2))
                nc.vector.tensor_scalar(out=ar_qk[:, :, 0], in0=ar_qk[:, :, 0],
                                        scalar1=1.0 / NCORES, scalar2=None,
                                        op0=OP.mult)
                nc.vector.tensor_scalar(out=ar_qk[:, :, 1], in0=ar_qk[:, :, 1],
                                        scalar1=1.0 / NCORES, scalar2=None,
                                        op0=OP.mult)
                nc.vector.tensor_tensor(out=m16[:, :], in0=ar_qk[:, :, 0],
                                        in1=ar_qk[:, :, 0], op=OP.mult)
                nc.vector.tensor_tensor(out=m16[:, :], in0=ar_qk[:, :, 1],
                                        in1=m16[:, :], op=OP.subtract)
                u16 = misc.tile([128, 16], F32)
                r16 = misc.tile([128, 16], F32)
                emit_rsqrt(m16[:, :], m16[:, :], u16[:, :], r16[:, :])
                nc.vector.tensor_tensor(out=qa[:, :], in0=m16[:, :],
                                        in1=gqk[:, :], op=OP.mult)
                nc.vector.tensor_tensor(out=qb[:, :], in0=ar_qk[:, :, 0],
                                        in1=qa[:, :], op=OP.mult)
                nc.vector.tensor_tensor(out=qb[:, :], in0=bqk[:, :],
                                        in1=qb[:, :], op=OP.subtract)
                for g in range(16):
                    nc.vector.tensor_scalar(
                        out=xa[g][:, :], in0=xa[g][:, :],
                        scalar1=qa[:, g:g + 1], scalar2=qb[:, g:g + 1],
                        op0=OP.mult, op1=OP.add)

            with tc.tile_pool(name=f"bc{rep}", bufs=1) as bcp:
                rstage = bcp.tile([128, T], F32)   # row 0: r, then 1/r
                # ---------------- P5: scores^T, exp, causal, row sums ----
                ae = {}
                scale = 1.0 / float(np.sqrt(C))
                with tc.tile_pool(name=f"psR{rep}", bufs=1, space="PSUM") as psR:
                    for tch in range(TQ):
                        acts = [st for st in range(NT) if 128 * st < 512 * (tch + 1)]
                        ps_r = psR.tile([1, 512], F32, tag=f"r{tch}",
                                        name=f"psr{tch}")
                        for ii, st in enumerate(acts):
                            ps = psA.tile([128, 512], F32, tag="mm",
                                          name=f"scps{tch}_{st}")
                            for j in range(8):
                                nc.tensor.matmul(
                                    ps[:, :],
                                    xa[8 + j][:, 128 * st:128 * (st + 1)],
                                    xa[j][:, 512 * tch:512 * (tch + 1)],
                                    start=(j == 0), stop=(j == 7))
                            a_t = attp.tile([128, 512], BF16, tag=f"ae{tch}_{st}",
                                            name=f"ae{tch}_{st}")
                            nc.scalar.activation(out=a_t[:, :], in_=ps[:, :],
                                                 func=ACTF.Exp, scale=scale)
                            base = 512 * tch - 128 * st
                            if base < 127:
                                nc.gpsimd.affine_select(
                                    out=a_t[:, :], in_=a_t[:, :],
                                    pattern=[[1, 512]], base=base,
                                    channel_multiplier=-1,
                                    compare_op=OP.is_ge, fill=0.0)
                            nc.tensor.matmul(ps_r[0:1, :], ones_b[:, :],
                                             a_t[:, :],
                                             start=(ii == 0),
                                             stop=(ii == len(acts) - 1))
                            ae[(tch, st)] = a_t
                        # chunked 1/r: reciprocal + broadcast per 512 tokens
                        rsl = rstage[0:1, 512 * tch:512 * (tch + 1)]
                        nc.vector.tensor_copy(out=rsl, in_=ps_r[0:1, :])
                        nc.vector.reciprocal(out=rsl, in_=rsl)
                        nc.gpsimd.dma_start(
                            out=rb_dram[512 * tch:512 * (tch + 1)], in_=rsl)
                        nc.gpsimd.dma_start(
                            out=r_bc[:, 512 * tch:512 * (tch + 1)],
                            in_=bcast_dram(rb_dram, 512 * tch, 512))

            # ---------------- v scale/bias math (post-AR2) ----
            gv8 = misc.tile([128, 8], F32)
            nc.sync.dma_start(out=gv8[:, :],
                              in_=gam[2048:3072].rearrange("(i p) -> p i", p=128))
            bv8 = misc.tile([128, 8], F32)
            nc.sync.dma_start(out=bv8[:, :],
                              in_=bet[2048:3072].rearrange("(i p) -> p i", p=128))
            vs_m = misc.tile([128, 8], F32)
            nc.sync.dma_start(out=vs_m[:, :],
                              in_=vstats_out[0:C].rearrange("(i p) -> p i", p=128))
            vs_e = misc.tile([128, 8], F32)
            nc.sync.dma_start(out=vs_e[:, :],
                              in_=vstats_out[C:2 * C].rearrange("(i p) -> p i",
                                                                p=128))
            m8 = misc.tile([128, 8], F32)
            va = misc.tile([128, 8], F32)
            vb = misc.tile([128, 8], F32)
            inv_n = 1.0 / (NCORES * T)
            nc.vector.tensor_scalar(out=vs_m[:, :], in0=vs_m[:, :],
                                    scalar1=inv_n, scalar2=None, op0=OP.mult)
            nc.vector.tensor_scalar(out=vs_e[:, :], in0=vs_e[:, :],
                                    scalar1=inv_n, scalar2=None, op0=OP.mult)
            nc.vector.tensor_tensor(out=m8[:, :], in0=vs_m[:, :],
                                    in1=vs_m[:, :], op=OP.mult)
            nc.vector.tensor_tensor(out=m8[:, :], in0=vs_e[:, :],
                                    in1=m8[:, :], op=OP.subtract)
            u8 = misc.tile([128, 8], F32)
            r8 = misc.tile([128, 8], F32)
            emit_rsqrt(m8[:, :], m8[:, :], u8[:, :], r8[:, :])
            nc.vector.tensor_tensor(out=va[:, :], in0=m8[:, :], in1=gv8[:, :],
                                    op=OP.mult)
            nc.vector.tensor_tensor(out=vb[:, :], in0=vs_m[:, :], in1=va[:, :],
                                    op=OP.mult)
            nc.vector.tensor_tensor(out=vb[:, :], in0=bv8[:, :], in1=vb[:, :],
                                    op=OP.subtract)

        # xapool closed (frees 64KB/partition for the projection weights)
        with (
            tc.tile_pool(name=f"projp{rep}", bufs=1) as projp,
            tc.tile_pool(name=f"psP{rep}", bufs=3, space="PSUM") as psP,
        ):
            # ---------------- P6: AV + fused 1/r + BN affine ----------------
            y = [None] * 8
            for tch in range(TQ):
                acts = [st for st in range(NT) if 128 * st < 512 * (tch + 1)]
                for ct in range(8):
                    ps = psA.tile([128, 512], F32, tag="mm",
                                  name=f"avps{tch}_{ct}")
                    for ii, st in enumerate(acts):
                        nc.tensor.matmul(
                            ps[:, :],
                            vnat[st][:, 128 * ct:128 * (ct + 1)],
                            ae[(tch, st)][:, :],
                            start=(ii == 0), stop=(ii == len(acts) - 1))
                    if y[ct] is None:
                        y[ct] = projp.tile([128, T], F32R, tag=f"y{ct}",
                                           name=f"y{ct}")
                    ysl = y[ct][:, 512 * tch:512 * (tch + 1)]
                    # Raw PSUM drain on Act so the bank frees without waiting
                    # on r/va/vb; 1/r (Pool) and the BN affine (DVE) follow
                    # in-place and only gate proj1, which has plenty of slack.
                    nc.scalar.copy(out=ysl, in_=ps[:, :])
                    nc.gpsimd.tensor_tensor(
                        out=ysl, in0=ysl,
                        in1=r_bc[:, 512 * tch:512 * (tch + 1)], op=OP.mult)
                    nc.vector.tensor_scalar(
                        out=ysl, in0=ysl,
                        scalar1=va[:, ct:ct + 1], scalar2=vb[:, ct:ct + 1],
                        op0=OP.mult, op1=OP.add)

            with tc.tile_pool(name=f"lorap2{rep}", bufs=1) as lorap2:
                lpa_sb = lorap2.tile([R, C], F32R)
                nc.sync.dma_start(out=lpa_sb[:, :], in_=lpaT[:, :])
                lpb_sb = lorap2.tile([R, C], F32R)
                nc.sync.dma_start(out=lpb_sb[:, :], in_=lpbB[:, :])

                wp = []
                wmp = []
                for ct in range(8):
                    w1 = projp.tile([128, C], F32R, tag=f"wp{ct}", name=f"wp{ct}")
                    nc.sync.dma_start(out=w1[:, :],
                                      in_=wpT[128 * ct:128 * (ct + 1), :])
                    wp.append(w1)
                    w2 = projp.tile([128, C], F32R, tag=f"wmp{ct}",
                                    name=f"wmp{ct}")
                    nc.sync.dma_start(out=w2[:, :],
                                      in_=wpT[128 * ct:128 * (ct + 1), :])
                    wmp.append(w2)
                for et in range(8):
                    for fc in range(2):
                        ps = psA.tile([128, 512], F32, tag="mm",
                                      name=f"dpps{et}_{fc}")
                        nc.tensor.matmul(
                            ps[:, :],
                            lpb_sb[:, 128 * et:128 * (et + 1)],
                            lpa_sb[:, 512 * fc:512 * (fc + 1)],
                            start=True, stop=True)
                        nc.vector.tensor_tensor(
                            out=wmp[et][:, 512 * fc:512 * (fc + 1)],
                            in0=wmp[et][:, 512 * fc:512 * (fc + 1)],
                            in1=ps[:, :], op=OP.add)

                # ---------------- P7: double projection ----------------
                y1 = [None] * 8
                for tch in range(TQ):
                    for et in range(8):
                        ps = psP.tile([128, 512], F32, tag="pp",
                                      name=f"p1ps{tch}_{et}")
                        for ct in range(8):
                            nc.tensor.matmul(
                                ps[:, :],
                                wp[ct][:, 128 * et:128 * (et + 1)],
                                y[ct][:, 512 * tch:512 * (tch + 1)],
                                start=(ct == 0), stop=(ct == 7))
                        if y1[et] is None:
                            y1[et] = projp.tile([128, T], F32R, tag=f"y1{et}",
                                                name=f"y1_{et}")
                        nc.scalar.copy(out=y1[et][:, 512 * tch:512 * (tch + 1)],
                                       in_=ps[:, :])
                for tch in range(TQ):
                    for ft in range(8):
                        ps = psP.tile([128, 512], F32, tag="pp",
                                      name=f"p2ps{tch}_{ft}")
                        for et in range(8):
                            nc.tensor.matmul(
                                ps[:, :],
                                wmp[et][:, 128 * ft:128 * (ft + 1)],
                                y1[et][:, 512 * tch:512 * (tch + 1)],
                                start=(et == 0), stop=(et == 7))
                        o_t = outst.tile([128, 512], F32, tag="o",
                                         name=f"o{tch}_{ft}")
                        nc.vector.tensor_copy(out=o_t[:, :], in_=ps[:, :])
                        nc.sync.dma_start(
                            out=out[128 * ft:128 * (ft + 1),
                                    512 * tch:512 * (tch + 1)],
                            in_=o_t[:, :])


_NC_CACHE = {}


def _get_nc(T):
    if T not in _NC_CACHE:
        _NC_CACHE[T] = build(T)
    return _NC_CACHE[T]


LAST_RESULTS = None
LAST_IN_MAPS = None


def kernel(x, W_attn, W_proj, lora_attn_A, lora_attn_B, lora_proj_A, lora_proj_B,
           bn_gamma, bn_beta):
    global LAST_RESULTS, LAST_IN_MAPS
    f = np.float32
    x = np.asarray(x, f)
    B, T, C_ = x.shape
    assert C_ == C and B == NCORES

    wT = np.ascontiguousarray(np.asarray(W_attn, f).T)      # [C, 3C]
    wpT = np.ascontiguousarray(np.asarray(W_proj, f).T)     # [C, C]
    laT = np.ascontiguousarray(np.asarray(lora_attn_A, f).T)   # [R, C]
    lbB = np.ascontiguousarray(np.asarray(lora_attn_B, f))     # [R, 3C]
    lpaT = np.ascontiguousarray(np.asarray(lora_proj_A, f).T)  # [R, C]
    lpbB = np.ascontiguousarray(np.asarray(lora_proj_B, f))    # [R, C]
    gam = np.ascontiguousarray(np.asarray(bn_gamma, f))
    bet = np.ascontiguousarray(np.asarray(bn_beta, f))

    in_maps = []
    for b in range(B):
        in_maps.append({
            "xT": np.ascontiguousarray(x[b].T),
            "wT": wT, "wpT": wpT, "laT": laT, "lbB": lbB,
            "lpaT": lpaT, "lpbB": lpbB, "gam": gam, "bet": bet,
        })

    LAST_IN_MAPS = in_maps
    nc = _get_nc(T)
    res = run_bass_kernel_spmd(nc, in_maps, core_ids=list(range(NCORES)))
    LAST_RESULTS = res
    return np.stack([np.asarray(res.results[b]["out"]).T for b in range(B)]).astype(f)
